# revision 28
# baseline (speedup 1.0000x reference)
"""CapsuleNetwork Trainium2 kernel — 8-core data parallel, transposed dataflow.

kernel(**inputs) takes FULL unsharded inputs (np arrays keyed as in
setup_inputs()) and returns the FULL [4096, 100] logits.

v3: HAM-warm software-pipelined routing. Per-class waves (4 matmuls) with a
3-deep PSUM t-tile rotation so the PE never head-blocks on the DVE product;
vt/phi produced in bursts; Wbar/Wcp resident; stage-A DMA batched 4 k-tiles
per transfer; keep-warm micro-matmuls across scalar-chain gaps.

Per-core dataflow (batch shard b=512; activations transposed [feature, batch];
class axis padded: class j=(4*jt+g) owns rows 32g..32g+16 of tile jt):
  x1T = relu(W1.T @ featT + b1)           bf16 matmuls, K padded to 10240
  x2T = relu(W2.T @ x1T + b2)             fp32r
  pT  = LayerNorm_i(WpFT.T @ x2T + bp)    grouped stats via indicator matmuls
  s0T = Wbar.T @ pn                       (iteration-0 uniform-softmax shortcut)
  2 x [squash-phi+vt; agreement -> ex; softmax+s], squash-phi+vt, logits.
"""
import sys

sys.path.insert(0, "/opt/trn_rl_repo")

from contextlib import ExitStack

import numpy as np
import ml_dtypes

import concourse.bass as bass
import concourse.tile as tile
from concourse import bacc, mybir
from concourse.bass_utils import run_bass_kernel_spmd

F32 = mybir.dt.float32
F32R = mybir.dt.float32r
BF16 = mybir.dt.bfloat16
FP8 = mybir.dt.float8e4

N_CORES = 8
B = 512            # per-core batch
ID = 10000
KPAD = 10240       # 80 * 128 = 20 * (4*128)
NK4 = 20           # groups of 4 k-tiles
H1, H2 = 512, 256
NP_, PD, NL, CD = 32, 8, 100, 16
NI = NP_ * PD      # 256
JP = 32
NJT = 25           # 25 tiles x [128 = 4 classes x 32]
PJ = NL * JP       # 3200
LN_EPS = 1e-5
SQ_EPS = 1e-8

_CACHE = {}


def _round_fp32r(x):
    """fp32 -> fp32r: RNE to 11 mantissa bits (matches walrus fp32_to_fp32r)."""
    u = np.ascontiguousarray(x, dtype=np.float32).view(np.uint32)
    r = (u.astype(np.uint64) + 0x7FF + ((u >> 12) & 1)) & 0xFFFFF000
    return np.ascontiguousarray(r.astype(np.uint32).view(np.float32))


def _f32(x):
    return np.ascontiguousarray(np.asarray(x, dtype=np.float32))


def _f32r(x):
    return _round_fp32r(np.asarray(x, dtype=np.float32))


def _bf16(x):
    return np.ascontiguousarray(
        np.asarray(x, dtype=np.float32).astype(ml_dtypes.bfloat16))


def _part(x, k):
    """[K, M] with K=k*128 -> SBUF-friendly [128, k, M]."""
    K, M = x.shape
    return np.ascontiguousarray(x.reshape(k, 128, M).transpose(1, 0, 2))


def host_prep(W1, b1, W2, b2, Wp, bp, ln_g, ln_b, Wr, Wc, bc):
    H = {}
    W1p = np.zeros((KPAD, H1), np.float32)
    W1p[:ID] = np.asarray(W1, dtype=np.float32)
    # [KPAD, H1] -> [20, 4, 128, H1] -> [20, 128, 4, H1]
    H["W1"] = _bf16(W1p.reshape(NK4, 4, 128, H1).transpose(0, 2, 1, 3))
    H["b1"] = _f32(np.asarray(b1).reshape(4, 128).T.reshape(128, 4, 1))
    H["W2"] = _f32r(_part(_f32(W2), 4))
    H["b2"] = _f32(np.asarray(b2).reshape(2, 128).T.reshape(128, 2, 1))
    H["WpFT"] = _f32r(_part(_f32(np.asarray(Wp).transpose(1, 0, 2).reshape(H2, NI)), 2))
    H["bpT"] = _f32(np.asarray(bp).reshape(2, 128).T.reshape(128, 2, 1))
    H["gT"] = _f32(np.asarray(ln_g).reshape(2, 128).T.reshape(128, 2, 1))
    H["lbT"] = _f32(np.asarray(ln_b).reshape(2, 128).T.reshape(128, 2, 1))
    Wr = np.asarray(Wr, dtype=np.float32)

    # Wbar (resident): [128 K(n,i) , 25 jt, 2 ch, 128 M(4j x 32)] with 1/NL folded
    Wbar = np.zeros((128, NJT, 2, 128), np.float32)
    # Wrs (resident bf16): [128 K(n,i), 2 ch, 25 jt, 128 M(4j x 32)]
    Wrs = np.zeros((128, 2, NJT, 128), np.float32)
    # Wrm (resident bf16): [128 K(4j x 32 o), 2 ch, 25 jt, 128 M(n,i)]
    Wrm = np.zeros((128, 2, NJT, 128), np.float32)
    for jt in range(NJT):
        for g in range(4):
            j = 4 * jt + g
            m_io = Wr[:, j].transpose(0, 2, 1).reshape(NI, CD)   # [(n,i), o]
            for ch in range(2):
                blk = m_io[128 * ch: 128 * (ch + 1)]             # [128, 16]
                Wbar[:, jt, ch, 32 * g: 32 * g + CD] = blk / NL
                Wrs[:, ch, jt, 32 * g: 32 * g + CD] = blk
                Wrm[32 * g: 32 * g + CD, ch, jt, :] = blk.T
    H["Wbar"] = _bf16(Wbar)
    H["Wrs"] = _bf16(Wrs)
    H["Wrm"] = _bf16(Wrm)

    E8m = np.zeros((NI, NP_), np.float32)    # mean over i (1/8 folded)
    Eexp8 = np.zeros((NP_, NI), np.float32)  # expand n -> (n,i)
    for n in range(NP_):
        E8m[n * PD: (n + 1) * PD, n] = 1.0 / PD
        Eexp8[n, n * PD: (n + 1) * PD] = 1.0
    H["E8m"] = _f32r(_part(E8m, 2))          # [128, 2, 32]
    H["Eexp8"] = _f32r(Eexp8)                # [32, 256]
    H["EI"] = _bf16(_part(Eexp8.T, 2))       # [128, 2, 32] sum over i (bf16)

    # EIe2: K=32 expansion pattern, strip-replicated:
    # EIe2[32*s + r, c, q] = 1 iff 16*c + (q >> 3) == r  (for each strip s)
    EIe2 = np.zeros((128, 2, 128), np.float32)
    for s in range(4):
        for c in range(2):
            for r in range(16):
                EIe2[32 * s + 16 * c + r, c, 8 * r: 8 * r + 8] = 1.0
    H["EIe2"] = _bf16(EIe2)

    # Esum: ex row 32g+n -> col n, all g (jt-independent)
    Esum = np.zeros((128, NP_), np.float32)
    for p in range(128):
        Esum[p, p % 32] = 1.0
    H["Esum"] = _bf16(Esum)

    Esq = np.zeros((128, NJT, NL), np.float32)
    Eexp32 = np.zeros((NL, NJT, 128), np.float32)
    for jt in range(NJT):
        for g in range(4):
            Esq[32 * g: 32 * g + CD, jt, 4 * jt + g] = 1.0
            Eexp32[4 * jt + g, jt, 32 * g: 32 * (g + 1)] = 1.0
    H["Esq"] = _bf16(Esq)
    H["Eexp32"] = _bf16(Eexp32)
    H["EI8"] = np.ascontiguousarray(
        H["EI"].astype(np.float32).astype(ml_dtypes.float8_e4m3))

    Wc = np.asarray(Wc, dtype=np.float32)
    # Wcp resident: [128 K(4j x 32, 16 used), 25 jt, 100]
    Wcp = np.zeros((128, NJT, NL), np.float32)
    for jt in range(NJT):
        for g in range(4):
            j = 4 * jt + g
            Wcp[32 * g: 32 * g + CD, jt, :] = Wc[j * CD: (j + 1) * CD]
    H["Wcp"] = _bf16(Wcp)
    H["bc"] = _f32(np.asarray(bc).reshape(NL, 1))
    H["ident"] = _f32(np.eye(128))
    return H


def build():
    nc = bacc.Bacc("TRN2", target_bir_lowering=False, debug=False)
    d = {}

    def din(name, shape, dt=F32):
        d[name] = nc.dram_tensor(name, list(shape), dt, kind="ExternalInput").ap()

    din("featT", (NK4, 128, 4, B), BF16)
    din("W1", (NK4, 128, 4, H1), BF16)
    din("b1", (128, 4, 1)); din("W2", (128, 4, H2), F32R); din("b2", (128, 2, 1))
    din("WpFT", (128, 2, NI), F32R)
    din("bpT", (128, 2, 1)); din("gT", (128, 2, 1)); din("lbT", (128, 2, 1))
    din("Wbar", (128, NJT, 2, 128), BF16)
    din("Wrs", (128, 2, NJT, 128), BF16)
    din("Wrm", (128, 2, NJT, 128), BF16)
    din("E8m", (128, 2, NP_), F32R); din("Eexp8", (NP_, NI), F32R)
    din("EI", (128, 2, NP_), BF16); din("EI8", (128, 2, NP_), FP8)
    din("EIe2", (128, 2, 128), BF16)
    din("Esq", (128, NJT, NL), BF16); din("Eexp32", (NL, NJT, 128), BF16)
    din("Esum", (128, NP_), BF16)
    din("Wcp", (128, NJT, NL), BF16); din("bc", (NL, 1))
    din("ident", (128, 128))
    out = nc.dram_tensor("logits", [B, NL], F32, kind="ExternalOutput").ap()

    AF = mybir.ActivationFunctionType
    OP = mybir.AluOpType

    with tile.TileContext(nc) as tc, ExitStack() as ctx:
        const = ctx.enter_context(tc.tile_pool(name="const", bufs=1))

        def cload(name, dt=F32):
            src = d[name]
            t = const.tile(list(src.shape), dt, tag=name)
            nc.sync.dma_start(t[:], src)
            return t

        b1_sb = cload("b1"); w2_sb = cload("W2", F32R); b2_sb = cload("b2")
        wpft_sb = cload("WpFT", F32R); bpt_sb = cload("bpT")
        gt_sb = cload("gT"); lbt_sb = cload("lbT")
        wbar_sb = cload("Wbar", BF16)
        wrs_sb = cload("Wrs", BF16); wrm_sb = cload("Wrm", BF16)
        e8m_sb = cload("E8m", F32R); eexp8_sb = cload("Eexp8", F32R)
        ei_sb = cload("EI", BF16); ei8_sb = cload("EI8", FP8)
        eie_sb = cload("EIe2", BF16)
        esq_sb = cload("Esq", BF16); eexp32_sb = cload("Eexp32", BF16)
        esum_sb = cload("Esum", BF16)
        wcp_sb = cload("Wcp", BF16)
        bc_sb = cload("bc"); ident_sb = cload("ident")
        eps_ln = const.tile([128, 1], F32, tag="epsln")
        nc.gpsimd.memset(eps_ln[:], LN_EPS)
        eps_sq = const.tile([128, 1], F32, tag="epssq")
        nc.gpsimd.memset(eps_sq[:], SQ_EPS)
        warm_mv = const.tile([16, 16], BF16, tag="warmmv")
        nc.gpsimd.memset(warm_mv[:], 0.5)

        state = ctx.enter_context(tc.tile_pool(name="state", bufs=1))
        x1T = state.tile([128, 4, B], F32R, tag="x1T")
        x2T = state.tile([128, 2, B], F32R, tag="x2T")
        pn = state.tile([128, 2, B], F32R, tag="pn")
        pn_b = state.tile([128, 2, B], BF16, tag="pn_b")
        pn_r = state.tile([128, 2, B], BF16, tag="pn_r")  # pn * (1/softmax-sum)
        sT = state.tile([128, NJT, B], BF16, tag="sT")
        ex = state.tile([128, NJT, B], BF16, tag="ex")
        vt = state.tile([128, NJT, B], BF16, tag="vt")
        sig = state.tile([NL, B], F32, tag="sig")
        phi = state.tile([NL, B], BF16, tag="phi")

        def mk_warm(pool):
            """Keep-warm dummy matmul emitter (N=512, reads stable x1T)."""
            wtile = pool.tile([NP_, B], F32, tag="warm")

            def warm(n=1):
                for _ in range(n):
                    nc.tensor.matmul(wtile[:], e8m_sb[:, 0, :], x1T[:, 0, :],
                                     start=True, stop=True)
            return warm

        # ---------- Stage A: x1T = relu(W1.T @ featT + b1) ----------
        with tc.tile_pool(name="psA", bufs=1, space="PSUM") as psA, \
             tc.tile_pool(name="streamA", bufs=8) as sa:
            x1_ps = [psA.tile([128, B], F32, tag=f"x1ps{h}", name=f"x1ps{h}")
                     for h in range(4)]

            def fetchA(g):
                # half-group: 2 k-tiles per DMA (g in 0..39)
                ft = sa.tile([128, 2, B], BF16, tag="ft")
                nc.sync.dma_start(ft[:], d["featT"][g // 2, :,
                                                    2 * (g % 2): 2 * (g % 2) + 2, :])
                wt = sa.tile([128, 2, H1], BF16, tag="wt")
                nc.sync.dma_start(wt[:], d["W1"][g // 2, :,
                                                 2 * (g % 2): 2 * (g % 2) + 2, :])
                return ft, wt

            NG = 2 * NK4
            pend = [fetchA(0), fetchA(1), fetchA(2), fetchA(3)]
            for g in range(NG):
                ft, wt = pend.pop(0)
                if g + 4 < NG:
                    pend.append(fetchA(g + 4))
                for kk in range(2):
                    for h in range(4):
                        nc.tensor.matmul(
                            x1_ps[h][:], wt[:, kk, bass.ts(h, 128)],
                            ft[:, kk, :],
                            start=(g == 0 and kk == 0),
                            stop=(g == NG - 1 and kk == 1))
            for h in range(4):
                nc.scalar.activation(x1T[:, h, :], x1_ps[h][:], AF.Relu,
                                     bias=b1_sb[:, h, :], scale=1.0)

        # ---------- Stage B + C + LayerNorm ----------
        with tc.tile_pool(name="psB", bufs=2, space="PSUM") as psB, \
             tc.tile_pool(name="psBa", bufs=1, space="PSUM") as psBa, \
             tc.tile_pool(name="scrB", bufs=2) as scrB:
            with tc.tile_pool(name="psW", bufs=1, space="PSUM") as psW:
                warm = mk_warm(psW)
                for c in range(2):
                    x2_ps = psB.tile([128, B], F32, tag="mm")
                    for k in range(4):
                        nc.tensor.matmul(
                            x2_ps[:], w2_sb[:, k, bass.ts(c, 128)], x1T[:, k, :],
                            start=(k == 0), stop=(k == 3))
                    nc.scalar.activation(x2T[:, c, :], x2_ps[:], AF.Relu,
                                         bias=b2_sb[:, c, :], scale=1.0)

                praw = scrB.tile([128, 2, B], F32R, tag="praw", bufs=1)
                for c in range(2):
                    p_ps = psB.tile([128, B], F32, tag="mm")
                    for k in range(2):
                        nc.tensor.matmul(
                            p_ps[:], wpft_sb[:, k, bass.ts(c, 128)], x2T[:, k, :],
                            start=(k == 0), stop=(k == 1))
                    nc.scalar.activation(praw[:, c, :], p_ps[:], AF.Identity,
                                         bias=bpt_sb[:, c, :], scale=1.0)
                warm()

                mu_ps = psBa.tile([NP_, B], F32, tag="acc")
                for c in range(2):
                    nc.tensor.matmul(mu_ps[:], e8m_sb[:, c, :], praw[:, c, :],
                                     start=(c == 0), stop=(c == 1))
                mu_sb = scrB.tile([NP_, B], F32R, tag="mu")
                nc.scalar.activation(mu_sb[:], mu_ps[:], AF.Copy, bias=0.0,
                                     scale=1.0)
                warm()

                q = scrB.tile([128, 2, B], F32, tag="q", bufs=1)
                var_ps = psBa.tile([NP_, B], F32, tag="acc2")
                for c in range(2):
                    me_ps = psB.tile([128, B], F32, tag="mm")
                    nc.tensor.matmul(me_ps[:], eexp8_sb[:, bass.ts(c, 128)],
                                     mu_sb[:], start=True, stop=True)
                    nc.vector.tensor_sub(q[:, c, :], praw[:, c, :].bitcast(F32),
                                         me_ps[:])
                    qsq = scrB.tile([128, B], F32R, tag="qsq")
                    nc.scalar.activation(qsq[:], q[:, c, :], AF.Square,
                                         bias=0.0, scale=1.0)
                    nc.tensor.matmul(var_ps[:], e8m_sb[:, c, :], qsq[:],
                                     start=(c == 0), stop=(c == 1))
                    warm()
                sd_sb = scrB.tile([NP_, B], F32R, tag="sd")
                nc.scalar.activation(sd_sb[:], var_ps[:], AF.Sqrt,
                                     bias=eps_ln[:NP_, :], scale=1.0)
                warm()
                for c in range(2):
                    se_ps = psB.tile([128, B], F32, tag="mm")
                    nc.tensor.matmul(se_ps[:], eexp8_sb[:, bass.ts(c, 128)],
                                     sd_sb[:], start=True, stop=True)
                    rstd_exp = scrB.tile([128, B], F32, tag="rstd")
                    nc.vector.reciprocal_approx_fast(rstd_exp[:], se_ps[:])
                    qr = scrB.tile([128, B], F32, tag="qr")
                    nc.vector.tensor_mul(qr[:], q[:, c, :], rstd_exp[:])
                    nc.scalar.activation(pn[:, c, :], qr[:], AF.Identity,
                                         bias=lbt_sb[:, c, :],
                                         scale=gt_sb[:, c, :])
                    nc.scalar.activation(pn_b[:, c, :],
                                         pn[:, c, :].bitcast(F32),
                                         AF.Copy, bias=0.0, scale=1.0)
                    warm()

        # ---------- s0 = Wbar.T @ pn (resident weights) ----------
        with tc.tile_pool(name="psS0", bufs=3, space="PSUM") as psS0, \
             tc.tile_pool(name="psW0", bufs=1, space="PSUM") as psW0:
            warm = mk_warm(psW0)
            warm(12)
            for jt in range(NJT):
                s_ps = psS0.tile([128, B], F32, tag="s")
                for c in range(2):
                    nc.tensor.matmul(s_ps[:], wbar_sb[:, jt, c, :],
                                     pn_b[:, c, :],
                                     start=(c == 0), stop=(c == 1))
                if jt % 2 == 0:
                    nc.scalar.activation(sT[:, jt, :], s_ps[:], AF.Copy,
                                         bias=0.0, scale=1.0)
                else:
                    nc.vector.tensor_copy(sT[:, jt, :], s_ps[:])

        # ---------- routing phase emitters ----------
        def emit_phi_vt():
            """sig = sum_o sT^2 -> phi; vt[:,jt,:] = phi_exp * sT (burst)."""
            with tc.tile_pool(name="psPhi", bufs=1, space="PSUM") as psPhi, \
                 tc.tile_pool(name="psE", bufs=2, space="PSUM") as psE, \
                 tc.tile_pool(name="scrP", bufs=3) as scrP, \
                 tc.tile_pool(name="psWp", bufs=1, space="PSUM") as psWp:
                warm = mk_warm(psWp)
                sig_ps = psPhi.tile([NL, B], F32, tag="sig")
                for jt in range(NJT):
                    sq = scrP.tile([128, B], BF16, tag="sq")
                    if jt % 2 == 0:
                        nc.vector.tensor_mul(sq[:], sT[:, jt, :], sT[:, jt, :])
                    else:
                        nc.scalar.activation(sq[:], sT[:, jt, :], AF.Square,
                                             bias=0.0, scale=1.0)
                    nc.tensor.matmul(sig_ps[:], esq_sb[:, jt, :], sq[:],
                                     start=(jt == 0), stop=(jt == NJT - 1))
                nc.scalar.activation(sig[:], sig_ps[:], AF.Copy, bias=0.0,
                                     scale=1.0)
                warm()
                u = scrP.tile([NL, B], F32, tag="u")
                nc.scalar.activation(u[:], sig[:], AF.Sqrt,
                                     bias=eps_sq[:NL, :], scale=1.0)
                warm()
                w = scrP.tile([NL, B], F32, tag="w")
                nc.vector.scalar_tensor_tensor(w[:], sig[:], 1.0, u[:],
                                               op0=OP.add, op1=OP.mult)
                warm()
                wr_ = scrP.tile([NL, B], F32, tag="wr")
                nc.vector.reciprocal_approx_fast(wr_[:], w[:])
                warm()
                nc.vector.tensor_mul(phi[:], sig[:], wr_[:])
                warm()
                # vt burst: 25 expand matmuls + trailing DVE muls
                pe_list = []
                for jt in range(NJT):
                    pe_ps = psE.tile([128, B], F32, tag="pe")
                    nc.tensor.matmul(pe_ps[:], eexp32_sb[:, jt, :], phi[:],
                                     start=True, stop=True)
                    pe_list.append((jt, pe_ps))
                    if len(pe_list) > 1:
                        pj, pp = pe_list.pop(0)
                        nc.vector.tensor_mul(vt[:, pj, :], pp[:], sT[:, pj, :])
                pj, pp = pe_list.pop(0)
                nc.vector.tensor_mul(vt[:, pj, :], pp[:], sT[:, pj, :])

        def emit_agreement(first):
            """blog += a(v): per-class waves in 2-wave batches, 4-engine pipe.

            wave (jt,g): PE-A: t = Wrm_g.T @ vt_g (2 MMs, [128,2B] PSUM)
                         EVAC (scalar/DVE alt): te2 = bf16(t)
                         MUL (DVE/gpsimd alt): prod2 = te2 * pn_b
                         PE-B (lag 4): a[g-strip] = EI-reduce(prod2) (2 MMs)
            """
            waves = [(jt, g) for jt in range(NJT) for g in range(4)]
            NW = len(waves)
            with tc.tile_pool(name="psT", bufs=3, space="PSUM") as psT, \
                 tc.tile_pool(name="psAg", bufs=2, space="PSUM") as psAg, \
                 tc.tile_pool(name="scrA", bufs=9) as scrA:
                t_tiles = {}
                te_tiles = {}
                prods = {}
                a_tiles = {}

                def emit_t2(ws):
                    # c-major across 3 waves: disjoint 32-row strips back to
                    # back so the PE sub-arrays overlap execution.
                    for w in ws:
                        t_tiles[w] = psT.tile([128, 2, B], F32, tag="t",
                                              name="t2")
                    for c in range(2):
                        for w in ws:
                            jt, g = waves[w]
                            nc.tensor.matmul(
                                t_tiles[w][:, c, :],
                                wrm_sb[32 * g: 32 * g + CD, c, jt, :],
                                vt[32 * g: 32 * g + CD, jt, :],
                                start=True, stop=True,
                                tile_position=(32 * g, 0))

                def emit_evac(w):
                    te2 = scrA.tile([128, 2, B], BF16, tag="te2")
                    if w % 6 == 5:
                        nc.vector.tensor_copy(te2[:], t_tiles.pop(w)[:])
                    else:
                        nc.scalar.activation(te2[:], t_tiles.pop(w)[:],
                                             AF.Copy, bias=0.0, scale=1.0)
                    te_tiles[w] = te2

                def emit_mul(w):
                    prod2 = scrA.tile([128, 2, B], BF16, tag="prod2")
                    src = te_tiles.pop(w)
                    r = w % 6
                    if r in (4, 5):
                        nc.gpsimd.tensor_mul(prod2[:], src[:], pn_b[:])
                    elif r == 3:
                        nc.gpsimd.tensor_mul(prod2[:, 0, :], src[:, 0, :],
                                             pn_b[:, 0, :])
                        nc.vector.tensor_mul(prod2[:, 1, :], src[:, 1, :],
                                             pn_b[:, 1, :])
                    else:
                        nc.vector.tensor_mul(prod2[:], src[:], pn_b[:])
                    prods[w] = prod2

                def emit_ei(ws):
                    for w in ws:
                        jt, g = waves[w]
                        if g == 0:
                            a_tiles[jt] = psAg.tile([128, B], F32, tag="a",
                                                    name="a_ps")
                    for c in range(2):
                        for w in ws:
                            jt, g = waves[w]
                            nc.tensor.matmul(
                                a_tiles[jt][32 * g: 32 * (g + 1), :],
                                ei_sb[:, c, :], prods[w][:, c, :],
                                start=(c == 0), stop=(c == 1),
                                tile_position=(0, 32 * g))
                    for w in ws:
                        jt, g = waves[w]
                        prods.pop(w)
                        if g == 3:
                            emit_ex(jt)

                def emit_ex(jt):
                    a_ps = a_tiles.pop(jt)
                    if first:
                        nc.scalar.activation(ex[:, jt, :], a_ps[:], AF.Exp,
                                             bias=0.0, scale=1.0)
                    else:
                        exf = scrA.tile([128, B], BF16, tag="exf")
                        nc.scalar.activation(exf[:], a_ps[:], AF.Exp,
                                             bias=0.0, scale=1.0)
                        nc.gpsimd.tensor_mul(ex[:, jt, :], ex[:, jt, :],
                                             exf[:])

                SG = 3
                NS = (NW + SG - 1) // SG
                for s in range(NS + 3):
                    cur = [w for w in range(SG * s, SG * s + SG) if w < NW]
                    if cur:
                        emit_t2(cur)
                        for w in cur:
                            emit_evac(w)
                    mw = [w for w in range(SG * (s - 2), SG * (s - 2) + SG)
                          if 0 <= w < NW]
                    for w in mw:
                        emit_mul(w)
                    ew = [w for w in range(SG * (s - 3), SG * (s - 3) + SG)
                          if 0 <= w < NW]
                    if ew:
                        emit_ei(ew)

        def emit_softmax_s():
            """c = softmax_j(blog); cp = c_exp*pn_r; sT = Wrs.T @ cp."""
            with tc.tile_pool(name="psZ", bufs=1, space="PSUM") as psZ, \
                 tc.tile_pool(name="psE2", bufs=2, space="PSUM") as psE2, \
                 tc.tile_pool(name="psWz", bufs=1, space="PSUM") as psWz, \
                 tc.tile_pool(name="scrS", bufs=3) as scrS:
                warm = mk_warm(psWz)
                sum_ps = psZ.tile([NP_, B], F32, tag="z")
                for jt in range(NJT):
                    nc.tensor.matmul(sum_ps[:], esum_sb[:], ex[:, jt, :],
                                     start=(jt == 0), stop=(jt == NJT - 1))
                sum_sb = scrS.tile([NP_, B], F32R, tag="sumsb")
                nc.scalar.activation(sum_sb[:], sum_ps[:], AF.Copy,
                                     bias=0.0, scale=1.0)
                warm(2)
                for c in range(2):
                    sne_ps = psE2.tile([128, B], F32, tag="sne")
                    nc.tensor.matmul(sne_ps[:], eexp8_sb[:, bass.ts(c, 128)],
                                     sum_sb[:], start=True, stop=True)
                    rni = scrS.tile([128, B], F32, tag="rni")
                    nc.vector.reciprocal_approx_fast(rni[:], sne_ps[:])
                    nc.vector.tensor_mul(pn_r[:, c, :],
                                         pn[:, c, :].bitcast(F32), rni[:])
                    warm(2)

            waves = [(jt, g) for jt in range(NJT) for g in range(4)]
            NW = len(waves)
            with tc.tile_pool(name="psC", bufs=3, space="PSUM") as psC, \
                 tc.tile_pool(name="psS", bufs=2, space="PSUM") as psS, \
                 tc.tile_pool(name="scrS2", bufs=9) as scrS2:
                ce_tiles = {}
                cee_tiles = {}
                cps = {}
                s_tiles = {}

                def emit_ce(ws):
                    for w in ws:
                        ce_tiles[w] = psC.tile([128, 2, B], F32, tag="ce",
                                               name="ce")
                    for c in range(2):
                        for w in ws:
                            jt, g = waves[w]
                            nc.tensor.matmul(
                                ce_tiles[w][:, c, :],
                                eie_sb[32 * g: 32 * (g + 1), c, :],
                                ex[32 * g: 32 * (g + 1), jt, :],
                                start=True, stop=True,
                                tile_position=(32 * g, 0))

                def emit_evac(w):
                    cee = scrS2.tile([128, 2, B], BF16, tag="cee")
                    if w % 6 == 5:
                        nc.vector.tensor_copy(cee[:], ce_tiles.pop(w)[:])
                    else:
                        nc.scalar.activation(cee[:], ce_tiles.pop(w)[:],
                                             AF.Copy, bias=0.0, scale=1.0)
                    cee_tiles[w] = cee

                def emit_cp(w):
                    cp2 = scrS2.tile([128, 2, B], BF16, tag="cp2")
                    src = cee_tiles.pop(w)
                    r = w % 6
                    if r in (4, 5):
                        nc.gpsimd.tensor_mul(cp2[:], src[:], pn_r[:])
                    elif r == 3:
                        nc.gpsimd.tensor_mul(cp2[:, 0, :], src[:, 0, :],
                                             pn_r[:, 0, :])
                        nc.vector.tensor_mul(cp2[:, 1, :], src[:, 1, :],
                                             pn_r[:, 1, :])
                    else:
                        nc.vector.tensor_mul(cp2[:], src[:], pn_r[:])
                    cps[w] = cp2

                def emit_sg(ws):
                    for w in ws:
                        jt, g = waves[w]
                        if g == 0:
                            s_tiles[jt] = psS.tile([128, B], F32, tag="s",
                                                   name="s_ps")
                    for c in range(2):
                        for w in ws:
                            jt, g = waves[w]
                            nc.tensor.matmul(
                                s_tiles[jt][32 * g: 32 * (g + 1), :],
                                wrs_sb[:, c, jt, 32 * g: 32 * (g + 1)],
                                cps[w][:, c, :],
                                start=(c == 0), stop=(c == 1),
                                tile_position=(0, 32 * g))
                    for w in ws:
                        jt, g = waves[w]
                        cps.pop(w)
                        if g == 3:
                            s_done = s_tiles.pop(jt)
                            if jt % 2 == 0:
                                nc.scalar.activation(sT[:, jt, :], s_done[:],
                                                     AF.Copy, bias=0.0,
                                                     scale=1.0)
                            else:
                                nc.vector.tensor_copy(sT[:, jt, :], s_done[:])

                SG = 3
                NS = (NW + SG - 1) // SG
                for s in range(NS + 3):
                    cur = [w for w in range(SG * s, SG * s + SG) if w < NW]
                    if cur:
                        emit_ce(cur)
                        for w in cur:
                            emit_evac(w)
                    mw = [w for w in range(SG * (s - 2), SG * (s - 2) + SG)
                          if 0 <= w < NW]
                    for w in mw:
                        emit_cp(w)
                    ew = [w for w in range(SG * (s - 3), SG * (s - 3) + SG)
                          if 0 <= w < NW]
                    if ew:
                        emit_sg(ew)

        # ---------- routing ----------
        emit_phi_vt()
        emit_agreement(first=True)
        emit_softmax_s()
        emit_phi_vt()
        emit_agreement(first=False)
        emit_softmax_s()
        emit_phi_vt()

        # ---------- logits ----------
        with tc.tile_pool(name="psL", bufs=1, space="PSUM") as psL, \
             tc.tile_pool(name="psTr", bufs=2, space="PSUM") as psTr, \
             tc.tile_pool(name="scrL", bufs=2) as scrL:
            lg_ps = psL.tile([NL, B], F32, tag="lg")
            for jt in range(NJT):
                nc.tensor.matmul(lg_ps[:], wcp_sb[:, jt, :], vt[:, jt, :],
                                 start=(jt == 0), stop=(jt == NJT - 1))
            lg_sb = scrL.tile([NL, B], F32, tag="lgsb")
            nc.scalar.activation(lg_sb[:], lg_ps[:], AF.Identity,
                                 bias=bc_sb[:], scale=1.0)
            for bt in range(4):
                tr_ps = psTr.tile([128, 128], F32, tag="tr")
                nc.tensor.transpose(tr_ps[:, :NL], lg_sb[:, bass.ts(bt, 128)],
                                    ident_sb[:NL, :NL])
                og = scrL.tile([128, NL], F32, tag="og")
                nc.vector.tensor_copy(og[:], tr_ps[:, :NL])
                nc.sync.dma_start(out[bass.ts(bt, 128), :], og[:])

    nc.compile()
    return nc


def kernel(features, W1, b1, W2, b2, Wp, bp, ln_g, ln_b, Wr, Wc, bc):
    features = np.ascontiguousarray(np.asarray(features, dtype=np.float32))
    H = host_prep(W1, b1, W2, b2, Wp, bp, ln_g, ln_b, Wr, Wc, bc)
    ins = {k: H[k] for k in [
        "W1", "b1", "W2", "b2", "WpFT", "bpT", "gT", "lbT", "Wbar", "Wrs",
        "Wrm", "E8m", "Eexp8", "EI", "EI8", "EIe2", "Esq", "Eexp32", "Esum",
        "Wcp", "bc", "ident"]}
    if "nc" not in _CACHE:
        _CACHE["nc"] = build()
    nc = _CACHE["nc"]

    in_maps = []
    for c in range(N_CORES):
        sl = features[c * B: (c + 1) * B]
        fT = np.zeros((KPAD, B), np.float32)
        fT[:ID] = sl.T
        in_maps.append({"featT": _bf16(
            fT.reshape(NK4, 4, 128, B).transpose(0, 2, 1, 3)), **ins})

    res = run_bass_kernel_spmd(nc, in_maps, list(range(N_CORES)))
    _CACHE["last_results"] = res
    return np.concatenate([res.results[c]["logits"] for c in range(N_CORES)],
                          axis=0)
